# revision 26
# baseline (speedup 1.0000x reference)
"""Trainium2 Bass kernel for Kaldi LinearResample (16 kHz -> 22.05 kHz) on [8, 960000].

out[b, 441*k + p] = sum_i x[b, 320*k - 6 + i] * B[i, p] with B the [384, 441]
polyphase tap matrix (13 taps/phase).  Pure data parallel: one row per core.

Layout: each SBUF partition holds FOUR consecutive 320-sample blocks plus
halo (1344 samples, 5376B DMA descriptors); outputs of the 4 blocks are
contiguous per partition (1764 f32 = 7056B descriptors).  Per q-tile of 128
partitions: one big DMA in -> 11 PE transposes (128x128 f32) -> DVE
copy-cast to bf16 -> 14 sparse accumulating bf16 matmuls against 7 unique
shifted filter tiles into 4 f32 PSUM banks (one per block phase) -> ACT
copies -> one contiguous DMA out.  Output DMAs alternate between the ACT
HWDGE ring and the gpsimd SWDGE path; inputs ride the SP HWDGE ring.
"""

import math

import numpy as np

N_IN = 960000
NK = 3000
P_PH = 441
IOFF = 6
QW = 1344          # samples per partition: 4*320 + 6 + 58
NQ = 750           # partitions (of 4 blocks each)
N_OUT = NK * P_PH
N_CORES = 8

# q-tiles: (first partition, partition count).  The last tile overlaps the
# previous by 2 partitions so every large DMA has a partition count that is
# a multiple of 16 (single-engine descriptor degeneration otherwise).
QTILES = [(0, 128), (128, 128), (256, 128), (384, 128), (512, 128), (638, 112)]

# per block-phase m: list of (transpose chunk index, unique-B-tile index)
USE = {0: [(0, 0), (1, 1), (2, 2)],
       1: [(2, 3), (3, 4), (4, 5), (5, 6)],
       2: [(5, 0), (6, 1), (7, 2)],
       3: [(7, 3), (8, 4), (9, 5), (10, 6)]}
# structural nonzero phase-column range per unique B tile
COLR = [(0, 176), (160, 352), (337, 441), (0, 87), (72, 264), (248, 440),
        (425, 441)]

_ORIG, _NEW, _LPW = 16000, 22050, 6


def _filter_tiles() -> np.ndarray:
    """[7, 128, 441] unique shifted filter tiles (f32)."""
    base = math.gcd(_ORIG, _NEW)
    P = _NEW // base
    cutoff = 0.99 * 0.5 * min(_ORIG, _NEW)
    ww = _LPW / (2.0 * cutoff)
    out_t = np.arange(P, dtype=np.float64) / _NEW
    min_i = np.ceil((out_t - ww) * _ORIG)
    max_i = np.floor((out_t + ww) * _ORIG)
    W = int((max_i - min_i + 1).max())
    j = np.arange(W, dtype=np.float64)
    inp_i = min_i[:, None] + j[None, :]
    dt = inp_i / _ORIG - out_t[:, None]
    w = np.zeros_like(dt)
    inside = np.abs(dt) < ww
    w[inside] = 0.5 * (1.0 + np.cos(2.0 * np.pi * cutoff / _LPW * dt[inside]))
    zero = dt == 0.0
    nz = ~zero
    w[nz] *= np.sin(2.0 * np.pi * cutoff * dt[nz]) / (np.pi * dt[nz])
    w[zero] *= 2.0 * cutoff
    w /= _ORIG
    fi = min_i.astype(np.int64)
    wf = w.astype(np.float32)
    Bfull = np.zeros((384, P), dtype=np.float32)
    for p in range(P):
        for jj in range(W):
            Bfull[fi[p] + IOFF + jj, p] += wf[p, jj]
    U = np.zeros((7, 128, P), dtype=np.float32)
    for c in range(3):                       # shift 0: chunks 0..2
        U[c] = Bfull[128 * c:128 * (c + 1)]
    for c in range(2, 6):                    # shift 320: chunks 2..5
        for r in range(128):
            src = 128 * c + r - 320
            if 0 <= src < 384:
                U[c + 1, r] = Bfull[src]
    return U


_CACHE: dict = {}


def _build():
    if "nc" in _CACHE:
        return _CACHE["nc"]

    import concourse.bass as bass
    import concourse.tile as tile
    from concourse import bacc, mybir

    F32 = mybir.dt.float32
    BF16 = mybir.dt.bfloat16

    nc = bacc.Bacc("TRN2", target_bir_lowering=False, debug=False,
                   num_devices=N_CORES)
    x_dram = nc.declare_dram_parameter("waveforms", [N_IN], F32, isOutput=False)
    b_dram = nc.declare_dram_parameter("bfilt", [128, 7 * P_PH], BF16,
                                       isOutput=False)
    i_dram = nc.declare_dram_parameter("ident", [128, 128], BF16, isOutput=False)
    o_dram = nc.declare_dram_parameter("out", [N_OUT], BF16, isOutput=True)
    xh = x_dram.ap().tensor
    bh = b_dram.ap().tensor
    oh = o_dram.ap().tensor

    with tile.TileContext(nc) as tc:
        with (
            tc.tile_pool(name="const", bufs=1) as cpool,
            tc.tile_pool(name="xin", bufs=5) as xpool,
            tc.tile_pool(name="xqt", bufs=3) as xtpool,
            tc.tile_pool(name="xqb", bufs=3) as xbpool,
            tc.tile_pool(name="pt", bufs=4, space="PSUM") as ptpool,
            tc.tile_pool(name="pacc", bufs=4, space="PSUM") as paccpool,
            tc.tile_pool(name="osb", bufs=3) as opool,
        ):
            # constants ride the (initially idle) ACT ring with large
            # descriptors; the SP ring starts the input stream immediately
            ident = cpool.tile([128, 128], BF16)
            nc.scalar.dma_start(ident[:], i_dram[:, :])
            bsb = cpool.tile([128, 7 * P_PH], BF16)
            nc.scalar.dma_start(bsb[:], b_dram[:, :])

            for qi in (1, 2, 5, 3, 0, 4):
                q0, nq = QTILES[qi]
                xq = xpool.tile([128, QW], F32)
                if qi == 0:
                    # partition 0 window starts at x[-6]; keep every large
                    # DMA at a multiple-of-16 partition count
                    nc.vector.memset(xq[0:32, 0:IOFF], 0.0)
                    nc.sync.dma_start(
                        xq[0:16, IOFF:QW],
                        bass.AP(xh, 0, [[1280, 16], [1, QW - IOFF]]),
                    )
                    nc.sync.dma_start(
                        xq[16:128, :],
                        bass.AP(xh, 1280 * 16 - IOFF, [[1280, 112], [1, QW]]),
                    )
                    nc.sync.dma_start(
                        xq[1:16, 0:IOFF],
                        bass.AP(xh, 1280 - IOFF, [[1280, 15], [1, IOFF]]),
                    )
                elif qi == len(QTILES) - 1:
                    # tile partitions are q 638..749; q=749 runs 58 past the
                    # input end (valid window: 1286 of 1344 samples)
                    vlast = N_IN - (1280 * (NQ - 1) - IOFF)  # 1286
                    nc.vector.memset(xq[96:128, vlast:QW], 0.0)
                    nc.sync.dma_start(
                        xq[0:96, :],
                        bass.AP(xh, 1280 * q0 - IOFF, [[1280, 96], [1, QW]]),
                    )
                    nc.sync.dma_start(
                        xq[96:112, 0:1280],
                        bass.AP(xh, 1280 * (q0 + 96) - IOFF, [[1280, 16], [1, 1280]]),
                    )
                    nc.sync.dma_start(
                        xq[96:111, 1280:QW],
                        bass.AP(xh, 1280 * (q0 + 96) - IOFF + 1280,
                                [[1280, 15], [1, QW - 1280]]),
                    )
                    nc.sync.dma_start(
                        xq[111:112, 1280:vlast],
                        bass.AP(xh, 1280 * (NQ - 1) - IOFF + 1280,
                                [[1280, 1], [1, vlast - 1280]]),
                    )
                else:
                    nc.sync.dma_start(
                        xq[:],
                        bass.AP(xh, 1280 * q0 - IOFF, [[1280, 128], [1, QW]]),
                    )

                # pre-cast so every PE LDWEIGHTS sees bf16 (enables fast
                # weight load throughout)
                xqb = xbpool.tile([128, QW], BF16)
                nc.vector.tensor_copy(xqb[:nq, :], xq[:nq, :])

                xqt = xtpool.tile([128, 11 * 128], BF16)
                for g in range(3):  # chunk quads (4, 4, 3)
                    gw = 4 if g < 2 else 3
                    pt = ptpool.tile([128, 4 * 128], BF16)
                    for j in range(gw):
                        c = 4 * g + j
                        cw = min(128, QW - 128 * c)  # 64 for chunk 10
                        nc.tensor.matmul(
                            pt[:cw, 128 * j:128 * j + nq],
                            xqb[:nq, 128 * c:128 * c + cw],
                            ident[:nq, :nq],
                            is_transpose=True,
                            skip_group_check=True,
                        )
                    nc.vector.tensor_copy(
                        xqt[:, 128 * 4 * g:128 * (4 * g + gw - 1) + nq],
                        pt[:, 0:128 * (gw - 1) + nq])

                ot = opool.tile([128, 4 * P_PH], BF16)
                for m in range(4):
                    pacc = paccpool.tile([128, P_PH], F32)
                    uses = USE[m]
                    for ui, (c, u) in enumerate(uses):
                        cw = min(128, QW - 128 * c)
                        c0, c1 = COLR[u]
                        nc.tensor.matmul(
                            pacc[:nq, c0:c1],
                            xqt[:cw, 128 * c:128 * c + nq],
                            bsb[:cw, P_PH * u + c0:P_PH * u + c1],
                            start=(ui == 0),
                            stop=(ui == len(uses) - 1),
                        )
                    nc.scalar.mul(ot[:nq, P_PH * m:P_PH * (m + 1)],
                                  pacc[:nq, :], 1.0)

                # output: 1764 contiguous f32 per partition
                # store per 2 phases (halves copy->store latency); paths
                # alternate between the ACT HWDGE ring and gpsimd SWDGE
                for h2 in range(2):
                    if qi == 5:
                        eng = nc.scalar if h2 == 0 else nc.gpsimd
                    else:
                        eng = nc.scalar if qi % 2 == 0 else nc.gpsimd
                    eng.dma_start(
                        bass.AP(oh, 4 * P_PH * q0 + 2 * P_PH * h2,
                                [[4 * P_PH, nq], [1, 2 * P_PH]]),
                        ot[:nq, 2 * P_PH * h2:2 * P_PH * (h2 + 1)],
                    )

    nc.compile()
    _CACHE["nc"] = nc
    return nc


def _run(waveforms: np.ndarray, trace: bool = False):
    import ml_dtypes

    from concourse.bass_utils import run_bass_kernel_spmd

    nc = _build()
    if "bmat" not in _CACHE:
        _CACHE["bmat"] = np.ascontiguousarray(
            _filter_tiles().transpose(1, 0, 2).reshape(128, 7 * P_PH)
        ).astype(ml_dtypes.bfloat16)
        _CACHE["ident"] = np.eye(128, dtype=np.float32).astype(ml_dtypes.bfloat16)
    bmat, idm = _CACHE["bmat"], _CACHE["ident"]
    in_maps = [
        {"waveforms": np.ascontiguousarray(waveforms[b], dtype=np.float32),
         "bfilt": bmat, "ident": idm}
        for b in range(N_CORES)
    ]
    last_err = None
    for attempt in range(3):
        try:
            res = run_bass_kernel_spmd(nc, in_maps, list(range(N_CORES)),
                                       trace=trace)
            out = np.stack([np.asarray(res.results[b]["out"]).reshape(N_OUT)
                            for b in range(N_CORES)]).astype(np.float32)
            return out, res
        except Exception as e:  # transient NRT device faults recover on retry
            last_err = e
            import time
            time.sleep(10)
    raise last_err


def kernel(waveforms: np.ndarray) -> np.ndarray:
    out, _ = _run(np.asarray(waveforms))
    return out
